# revision 4
# baseline (speedup 1.0000x reference)
"""AttnBlockST Trainium2 kernel (v2: folded weights + fp8 DoubleRow).

Two SPMD phases on 8 NeuronCores:
  phase 1 (spatial): data-parallel over b*t (32 samples -> 4/core),
    attention over hw=1024 within each (bt, c, hw) sample.
  phase 2 (temporal): data-parallel over b*h*w (2048 -> 256/core),
    attention over t=16, 8 samples packed per 128-partition group with a
    block-diagonal logit mask.

Algebraic folding (host side):
  logits  = q.k  with q = Wq hn + bq, k = Wk hn + bk.  Terms constant
  over the softmax axis drop, so S = g.hn with a single projection
  g = (Wq_e^T Wk_e) hn + Wk_e^T bq_e.
  output  = Wo (V P) + bo with V = Wv hn + bv and P columns summing to 1,
  so out = (Wo Wv) hn P + (Wo bv + bo): a single projection v' = N hn.
GroupNorm affine is folded into the weights as well.  Matmuls run in
fp8e4 (DoubleRow, fp32 PSUM accumulate) with power-of-2 scales; softmax
and GN statistics are fp32.
"""

import numpy as np
import ml_dtypes
from contextlib import ExitStack

import concourse.bass as bass
import concourse.mybir as mybir
import concourse.tile as tile
from concourse.bass_utils import run_bass_kernel_spmd

# ---- walrus workaround: split multi-wait final drain ----
from concourse.vector_clock import ScopedClock
from concourse.tile import TileContext


def _patched_drain_and_barrier(self, tick_clock, wait_clock):
    nc = self.nc
    drain_inst = nc.sync.drain()
    wait_clock.add_sem_waits(
        drain_inst.ins, ScopedClock({None: tick_clock.global_clock})
    )
    si = drain_inst.ins.sync_info
    if si is not None and len(si.on_wait) > 1:
        waits = list(si.on_wait)
        drain_inst.ins.sync_info = mybir.SyncInfo(
            on_wait=waits[:1], on_update=list(si.on_update)
        )
        for w in waits[1:]:
            n = nc.sync.nop(nofuse=True, hint="drain_wait_split")
            n.ins.sync_info = mybir.SyncInfo(on_wait=[w], on_update=[])
    nc.all_engine_barrier()
    assert self.sems is not None
    popped = nc._tile_sem_poison_stack.pop()
    assert popped is self._sem_poison
    nc.clear_and_free_semaphores(list(self.sems.allocated().values()))
    nc.all_engine_barrier()


TileContext._drain_and_barrier = _patched_drain_and_barrier

# ---- problem constants (hardcoded per spec) ----
B, C, T, H, W = 2, 512, 16, 32, 32
GROUPS = 32
EPS = 1e-6
N_CORES = 8
P = 128
CCH = C // P          # 4 channel chunks
KP = CCH // 2         # 2 DoubleRow channel-chunk pairs
GPC = GROUPS // CCH   # 8 groups per 128-channel chunk
GS = C // GROUPS      # 16 channels per group

L1 = H * W            # 1024 spatial positions
NS1 = (B * T) // N_CORES   # 4 samples per core, phase 1
LCH1 = L1 // P        # 8 position chunks

NT2 = 16              # temporal length
NS2 = (B * H * W) // N_CORES  # 256 samples per core, phase 2
HALF = NS2 // 2       # process in halves of 128 samples
F2 = HALF * NT2       # 2048 free columns per half
NGRP = F2 // P        # 16 groups of 8 samples per half

SG = 2048.0           # fp8 scale on the S-projection weights
SV = 32.0             # fp8 scale on the V-projection weights
SDG = 4.0             # fp8 scale on the phase-2 softmax normalizer diag

F32 = mybir.dt.float32
BF16 = mybir.dt.bfloat16
F8 = mybir.dt.float8e4
AX = mybir.AxisListType.X
AF = mybir.ActivationFunctionType
DR = mybir.MatmulPerfMode.DoubleRow

BF16NP = ml_dtypes.bfloat16
F8NP = ml_dtypes.float8_e4m3


def _op():
    from concourse.alu_op_type import AluOpType
    return AluOpType


def _bcast_inner(ap, n):
    """View (P, F) access pattern as (P, F, n) with stride-0 inner dim."""
    return bass.AP(tensor=ap.tensor, offset=ap.offset, ap=list(ap.ap) + [[0, n]])


def _split_waits(nc, limit=1):
    """This walrus build rejects >1 sem wait on every ISA template tested
    (LDWEIGHTS, CTRL, ACT, DVE TensorScalar); hoist extra waits onto
    same-engine NoOps placed just before."""
    ctr = [0]
    for f in nc.m.functions:
        for b in f.blocks:
            new = []
            for ins in b.instructions:
                si = getattr(ins, "sync_info", None)
                waits = list(si.on_wait) if si is not None and si.on_wait else []
                lim = limit
                if len(waits) > lim:
                    for w in waits[lim:]:
                        ctr[0] += 1
                        new.append(mybir.InstNoOp(
                            name=f"wsplit-{ctr[0]}",
                            sync_info=mybir.SyncInfo(on_wait=[w], on_update=[]),
                            bass_nofuse=True,
                            engine=ins.engine,
                        ))
                    ins.sync_info = mybir.SyncInfo(
                        on_wait=waits[:lim], on_update=list(si.on_update)
                    )
                new.append(ins)
            b.instructions = new
    return nc


# ---------------------------------------------------------------- phase 1
def build_spatial(reps=1):
    nc = bass.Bass()
    xs = nc.dram_tensor("xs", [NS1, C, L1], BF16, kind="ExternalInput")
    ys = nc.dram_tensor("ys", [NS1, C, L1], BF16, kind="ExternalOutput")
    wg_d = nc.dram_tensor("wg", [P, KP, 2, C], F8, kind="ExternalInput")
    wv_d = nc.dram_tensor("wv", [P, KP, 2, C], F8, kind="ExternalInput")
    ug_d = nc.dram_tensor("ug", [P, CCH], F32, kind="ExternalInput")
    bo_d = nc.dram_tensor("bo", [P, CCH], F32, kind="ExternalInput")
    gmask_d = nc.dram_tensor("gmask", [P, GPC], F32, kind="ExternalInput")
    bmask_d = nc.dram_tensor("bmask", [GPC, P], F32, kind="ExternalInput")
    A = _op()

    with tile.TileContext(nc) as tc, ExitStack() as ctx:
        ctx.enter_context(nc.allow_low_precision(
            reason="16-bit softmax normalizers are within the 2e-2 tolerance"))
        const = ctx.enter_context(tc.tile_pool(name="const", bufs=1))
        stp = ctx.enter_context(tc.tile_pool(name="stats", bufs=4))
        xp = ctx.enter_context(tc.tile_pool(name="x", bufs=2))
        hp = ctx.enter_context(tc.tile_pool(name="h", bufs=2))
        gp = ctx.enter_context(tc.tile_pool(name="g", bufs=2))
        vp = ctx.enter_context(tc.tile_pool(name="v", bufs=2))
        ep = ctx.enter_context(tc.tile_pool(name="est", bufs=2))
        rp = ctx.enter_context(tc.tile_pool(name="rcb", bufs=2))
        tp = ctx.enter_context(tc.tile_pool(name="t", bufs=3))
        yp = ctx.enter_context(tc.tile_pool(name="y", bufs=3))
        psm = ctx.enter_context(tc.tile_pool(name="psm", bufs=6, space="PSUM"))
        psg = ctx.enter_context(tc.tile_pool(name="psg", bufs=2, space="PSUM"))

        wg = const.tile([P, KP, 2, C], F8, tag="wg")
        nc.sync.dma_start(out=wg, in_=wg_d[:, :, :, :])
        wv = const.tile([P, KP, 2, C], F8, tag="wv")
        nc.sync.dma_start(out=wv, in_=wv_d[:, :, :, :])
        ug = const.tile([P, CCH], F32, tag="ug")
        nc.sync.dma_start(out=ug, in_=ug_d[:, :])
        bo = const.tile([P, CCH], F32, tag="bo")
        nc.sync.dma_start(out=bo, in_=bo_d[:, :])
        gmask = const.tile([P, GPC], F32, tag="gmask")
        nc.sync.dma_start(out=gmask, in_=gmask_d[:, :])
        bmask = const.tile([GPC, P], F32, tag="bmask")
        nc.sync.dma_start(out=bmask, in_=bmask_d[:, :])
        eps_t = const.tile([GPC, 1], F32, tag="eps")
        nc.vector.memset(eps_t, EPS)
        ones_rs = const.tile([P, 2, 16], F8, tag="ones_rs")
        nc.vector.memset(ones_rs, 1.0)
        ones_bc = const.tile([1, P], BF16, tag="ones_bc")
        nc.vector.memset(ones_bc, 1.0 / SV)

        for i_rep in range(reps * NS1):
            i = i_rep % NS1
            x_sb = xp.tile([P, CCH, L1], BF16)
            nc.sync.dma_start(out=x_sb, in_=xs[i].rearrange("(k p) l -> p k l", p=P))

            # ---- GroupNorm -> hn (fp8) ----
            h_sb = hp.tile([P, CCH, L1], F8, tag="h")
            for k in range(CCH):
                xc = x_sb[:, k, :]
                st = stp.tile([P, 2, 6], F32, tag="bnst")
                nc.vector.bn_stats(out=st[:, 0, :], in_=xc[:, 0:512])
                nc.vector.bn_stats(out=st[:, 1, :], in_=xc[:, 512:1024])
                mv = stp.tile([P, 2], F32, tag="mv")
                nc.vector.bn_aggr(out=mv, in_=st)
                me = stp.tile([P, 2], F32, tag="me")
                nc.vector.tensor_copy(out=me[:, 0:1], in_=mv[:, 0:1])
                m2 = stp.tile([P, 1], F32, tag="m2")
                nc.vector.tensor_mul(out=m2, in0=mv[:, 0:1], in1=mv[:, 0:1])
                nc.vector.tensor_add(out=me[:, 1:2], in0=mv[:, 1:2], in1=m2)
                gs_ps = psg.tile([GPC, 2], F32, tag="gn")
                nc.tensor.matmul(out=gs_ps, lhsT=gmask, rhs=me, start=True, stop=True)
                gs = stp.tile([GPC, 2], F32, tag="gs")
                nc.vector.tensor_copy(out=gs, in_=gs_ps)
                var = stp.tile([GPC, 1], F32, tag="var")
                nc.vector.tensor_mul(out=var, in0=gs[:, 0:1], in1=gs[:, 0:1])
                var2 = stp.tile([GPC, 1], F32, tag="var2")
                nc.vector.tensor_sub(out=var2, in0=gs[:, 1:2], in1=var)
                sd = stp.tile([GPC, 1], F32, tag="sd")
                nc.scalar.activation(out=sd, in_=var2, func=AF.Sqrt, bias=eps_t)
                ab = stp.tile([GPC, 2], F32, tag="ab")
                nc.vector.reciprocal(out=ab[:, 0:1], in_=sd)
                nc.vector.scalar_tensor_tensor(
                    out=ab[:, 1:2], in0=gs[:, 0:1], scalar=-1.0, in1=ab[:, 0:1],
                    op0=A.mult, op1=A.mult,
                )
                abc_ps = psg.tile([P, 2], F32, tag="gn")
                nc.tensor.matmul(out=abc_ps, lhsT=bmask, rhs=ab, start=True, stop=True)
                abc = stp.tile([P, 2], F32, tag="abc")
                nc.vector.tensor_copy(out=abc, in_=abc_ps)
                nc.vector.tensor_scalar(
                    out=h_sb[:, k, :], in0=xc,
                    scalar1=abc[:, 0:1], scalar2=abc[:, 1:2],
                    op0=A.mult, op1=A.add,
                )

            # ---- g projection: g = (SG*Mq^T Mk) hn + SG*u  (fp8) ----
            g_sb = gp.tile([P, CCH, L1], F8, tag="g")
            for m in range(CCH):
                for nb in range(2):
                    ps = psm.tile([P, 512], F32, tag="mm")
                    for kp in range(KP):
                        nc.tensor.matmul(
                            out=ps,
                            lhsT=wg[:, kp, :, m * P:(m + 1) * P],
                            rhs=h_sb[:, 2 * kp:2 * kp + 2, nb * 512:(nb + 1) * 512],
                            start=(kp == 0), stop=(kp == KP - 1),
                            perf_mode=DR,
                        )
                    nc.any.tensor_scalar_add(
                        out=g_sb[:, m, nb * 512:(nb + 1) * 512], in0=ps,
                        scalar1=ug[:, m:m + 1],
                    )

            # ---- ST = hn^T g (keys on partitions), exp -> est (fp8) ----
            est = ep.tile([P, LCH1, L1], F8, tag="est")
            for jb in range(LCH1):
                for nb in range(2):
                    ps = psm.tile([P, 512], F32, tag="mm")
                    for kp in range(KP):
                        nc.tensor.matmul(
                            out=ps,
                            lhsT=h_sb[:, 2 * kp:2 * kp + 2, jb * P:(jb + 1) * P],
                            rhs=g_sb[:, 2 * kp:2 * kp + 2, nb * 512:(nb + 1) * 512],
                            start=(kp == 0), stop=(kp == KP - 1),
                            perf_mode=DR,
                        )
                    nc.scalar.activation(
                        out=est[:, jb, nb * 512:(nb + 1) * 512], in_=ps,
                        func=AF.Exp, scale=1.0 / SG,
                    )

            # ---- v' = (SV*Wo Wv) hn, positions on partitions (fp8) ----
            vT_sb = vp.tile([P, LCH1, C], F8, tag="v")
            for pb in range(LCH1):
                ps = psm.tile([P, 512], F32, tag="mm")
                for kp in range(KP):
                    nc.tensor.matmul(
                        out=ps,
                        lhsT=h_sb[:, 2 * kp:2 * kp + 2, pb * P:(pb + 1) * P],
                        rhs=wv[:, kp, :, :],
                        start=(kp == 0), stop=(kp == KP - 1),
                        perf_mode=DR,
                    )
                nc.any.tensor_copy(out=vT_sb[:, pb, :], in_=ps)

            # ---- rs = column sums of est; rcb = (1/(SV*rs)) broadcast ----
            rc_bf = stp.tile([1, L1], BF16, tag="rc")
            for nb in range(2):
                prs = psm.tile([1, 512], F32, tag="mm")
                for a in range(LCH1 // 2):
                    nc.tensor.matmul(
                        out=prs,
                        lhsT=ones_rs[:, :, 0:1],
                        rhs=est[:, 2 * a:2 * a + 2, nb * 512:(nb + 1) * 512],
                        start=(a == 0), stop=(a == LCH1 // 2 - 1),
                        perf_mode=DR,
                    )
                nc.vector.reciprocal(
                    out=rc_bf[:, nb * 512:(nb + 1) * 512], in_=prs
                )
            rcb = rp.tile([P, L1], BF16, tag="rcb")
            for nb in range(2):
                prc = psm.tile([P, 512], F32, tag="mm")
                nc.tensor.matmul(
                    out=prc, lhsT=ones_bc,
                    rhs=rc_bf[:, nb * 512:(nb + 1) * 512],
                    start=True, stop=True,
                )
                nc.any.tensor_copy(out=rcb[:, nb * 512:(nb + 1) * 512], in_=prc)

            # ---- O = v' est, normalize, + bias + x -> ys ----
            for mc in range(CCH):
                for nb in range(2):
                    ps = psm.tile([P, 512], F32, tag="mm")
                    for a in range(LCH1 // 2):
                        nc.tensor.matmul(
                            out=ps,
                            lhsT=vT_sb[:, 2 * a:2 * a + 2, mc * P:(mc + 1) * P],
                            rhs=est[:, 2 * a:2 * a + 2, nb * 512:(nb + 1) * 512],
                            start=(a == 0), stop=(a == LCH1 // 2 - 1),
                            perf_mode=DR,
                        )
                    t_sb = tp.tile([P, 512], BF16, tag="t")
                    nc.vector.tensor_tensor(
                        out=t_sb, in0=ps,
                        in1=rcb[:, nb * 512:(nb + 1) * 512], op=A.mult,
                    )
                    y_sb = yp.tile([P, 512], BF16, tag="y")
                    nc.vector.scalar_tensor_tensor(
                        out=y_sb, in0=t_sb, scalar=bo[:, mc:mc + 1],
                        in1=x_sb[:, mc, nb * 512:(nb + 1) * 512],
                        op0=A.add, op1=A.add,
                    )
                    nc.sync.dma_start(
                        out=ys[i, mc * P:(mc + 1) * P,
                               nb * 512:(nb + 1) * 512],
                        in_=y_sb,
                    )
    return nc


# ---------------------------------------------------------------- phase 2
def build_temporal(reps=1):
    nc = bass.Bass()
    xt = nc.dram_tensor("xt", [C, NS2 * NT2], BF16, kind="ExternalInput")
    yt = nc.dram_tensor("yt", [C, NS2 * NT2], F32, kind="ExternalOutput")
    wg_d = nc.dram_tensor("wg", [P, KP, 2, C], F8, kind="ExternalInput")
    wv_d = nc.dram_tensor("wv", [P, KP, 2, C], F8, kind="ExternalInput")
    ug_d = nc.dram_tensor("ug", [P, CCH], F32, kind="ExternalInput")
    bo_d = nc.dram_tensor("bo", [P, CCH], F32, kind="ExternalInput")
    gmask_d = nc.dram_tensor("gmask", [P, GPC], BF16, kind="ExternalInput")
    bmask_d = nc.dram_tensor("bmask", [GPC, P], BF16, kind="ExternalInput")
    blkmask_d = nc.dram_tensor("blkmask", [P, P], F32, kind="ExternalInput")
    ident_d = nc.dram_tensor("ident", [P, P], F8, kind="ExternalInput")
    A = _op()
    NN = HALF

    with tile.TileContext(nc) as tc, ExitStack() as ctx:
        ctx.enter_context(nc.allow_low_precision(
            reason="16-bit GN stats are within the 2e-2 tolerance"))
        const = ctx.enter_context(tc.tile_pool(name="const", bufs=1))
        stp = ctx.enter_context(tc.tile_pool(name="stats", bufs=4))
        xp = ctx.enter_context(tc.tile_pool(name="x", bufs=2))
        sqp = ctx.enter_context(tc.tile_pool(name="sq", bufs=2))
        hp = ctx.enter_context(tc.tile_pool(name="h", bufs=2))
        gp = ctx.enter_context(tc.tile_pool(name="g", bufs=2))
        vp = ctx.enter_context(tc.tile_pool(name="v", bufs=2))
        xbp = ctx.enter_context(tc.tile_pool(name="xb", bufs=2))
        pp = ctx.enter_context(tc.tile_pool(name="pm", bufs=3))
        yp = ctx.enter_context(tc.tile_pool(name="y", bufs=3))
        psm = ctx.enter_context(tc.tile_pool(name="psm", bufs=6, space="PSUM"))
        psg = ctx.enter_context(tc.tile_pool(name="psg", bufs=2, space="PSUM"))

        wg = const.tile([P, KP, 2, C], F8, tag="wg")
        nc.sync.dma_start(out=wg, in_=wg_d[:, :, :, :])
        wv = const.tile([P, KP, 2, C], F8, tag="wv")
        nc.sync.dma_start(out=wv, in_=wv_d[:, :, :, :])
        ug = const.tile([P, CCH], F32, tag="ug")
        nc.sync.dma_start(out=ug, in_=ug_d[:, :])
        bo = const.tile([P, CCH], F32, tag="bo")
        nc.sync.dma_start(out=bo, in_=bo_d[:, :])
        gmask = const.tile([P, GPC], BF16, tag="gmask")
        nc.sync.dma_start(out=gmask, in_=gmask_d[:, :])
        bmask = const.tile([GPC, P], BF16, tag="bmask")
        nc.sync.dma_start(out=bmask, in_=bmask_d[:, :])
        blkmask = const.tile([P, P], F32, tag="blkmask")
        nc.sync.dma_start(out=blkmask, in_=blkmask_d[:, :])
        ident = const.tile([P, P], F8, tag="ident")
        nc.sync.dma_start(out=ident, in_=ident_d[:, :])
        eps_t = const.tile([GPC, 1], F32, tag="eps")
        nc.vector.memset(eps_t, EPS)

        xr = xt.rearrange("(k p) f -> p k f", p=P)
        yr = yt.rearrange("(k p) f -> p k f", p=P)

        for ih_rep in range(reps * 2):
            ih = ih_rep % 2
            f0 = ih * F2
            x_sb = xp.tile([P, CCH, F2], BF16)
            nc.sync.dma_start(out=x_sb, in_=xr[:, :, f0:f0 + F2])

            # ---- GroupNorm over (16c x 16t) per sample -> hn (fp8) ----
            h_sb = hp.tile([P, CCH, F2], F8, tag="h")
            for k in range(CCH):
                xc = x_sb[:, k, :]
                xc3 = x_sb[:, k, :].rearrange("p (n t) -> p n t", t=NT2)
                sq = sqp.tile([P, F2], BF16, tag="sq")
                nc.scalar.activation(out=sq, in_=xc, func=AF.Square)
                me = stp.tile([P, 2, NN], BF16, tag="me2")
                nc.vector.reduce_sum(out=me[:, 0, :], in_=xc3, axis=AX)
                nc.vector.reduce_sum(
                    out=me[:, 1, :],
                    in_=sq.rearrange("p (n t) -> p n t", t=NT2), axis=AX,
                )
                gs_ps = psg.tile([GPC, 2, NN], F32, tag="gn")
                nc.tensor.matmul(
                    out=gs_ps.rearrange("g a n -> g (a n)"),
                    lhsT=gmask, rhs=me.rearrange("p a n -> p (a n)"),
                    start=True, stop=True,
                )
                gs = stp.tile([GPC, 2, NN], F32, tag="gs2")
                nc.vector.tensor_copy(out=gs, in_=gs_ps)
                var = stp.tile([GPC, NN], F32, tag="var2a")
                nc.vector.tensor_mul(out=var, in0=gs[:, 0, :], in1=gs[:, 0, :])
                var2 = stp.tile([GPC, NN], F32, tag="var2b")
                nc.vector.tensor_sub(out=var2, in0=gs[:, 1, :], in1=var)
                sd = stp.tile([GPC, NN], F32, tag="sd2")
                nc.scalar.activation(out=sd, in_=var2, func=AF.Sqrt, bias=eps_t)
                ab = stp.tile([GPC, 2, NN], BF16, tag="ab2")
                nc.vector.reciprocal(out=ab[:, 0, :], in_=sd)
                nc.vector.scalar_tensor_tensor(
                    out=ab[:, 1, :], in0=gs[:, 0, :], scalar=-1.0, in1=ab[:, 0, :],
                    op0=A.mult, op1=A.mult,
                )
                abc_ps = psg.tile([P, 2, NN], F32, tag="gn")
                nc.tensor.matmul(
                    out=abc_ps.rearrange("p a n -> p (a n)"),
                    lhsT=bmask, rhs=ab.rearrange("g a n -> g (a n)"),
                    start=True, stop=True,
                )
                abc = stp.tile([P, 2, NN], F32, tag="abc2")
                nc.vector.tensor_copy(out=abc, in_=abc_ps)
                tmp = sqp.tile([P, F2], BF16, tag="tmp")
                nc.vector.tensor_tensor(
                    out=tmp.rearrange("p (n t) -> p n t", t=NT2),
                    in0=xc3, in1=_bcast_inner(abc[:, 0, :], NT2), op=A.mult,
                )
                nc.vector.tensor_tensor(
                    out=h_sb[:, k, :].rearrange("p (n t) -> p n t", t=NT2),
                    in0=tmp.rearrange("p (n t) -> p n t", t=NT2),
                    in1=_bcast_inner(abc[:, 1, :], NT2), op=A.add,
                )

            # ---- g projection (fp8 DoubleRow) ----
            g_sb = gp.tile([P, CCH, F2], F8, tag="g")
            for m in range(CCH):
                for nb in range(4):
                    ps = psm.tile([P, 512], F32, tag="mm")
                    for kp in range(KP):
                        nc.tensor.matmul(
                            out=ps,
                            lhsT=wg[:, kp, :, m * P:(m + 1) * P],
                            rhs=h_sb[:, 2 * kp:2 * kp + 2, nb * 512:(nb + 1) * 512],
                            start=(kp == 0), stop=(kp == KP - 1),
                            perf_mode=DR,
                        )
                    nc.any.tensor_scalar_add(
                        out=g_sb[:, m, nb * 512:(nb + 1) * 512], in0=ps,
                        scalar1=ug[:, m:m + 1],
                    )

            # ---- v' (positions on partitions, fp8 DoubleRow) ----
            vT_sb = vp.tile([P, NGRP, C], F8, tag="v")
            for pb in range(NGRP):
                ps = psm.tile([P, 512], F32, tag="mm")
                for kp in range(KP):
                    nc.tensor.matmul(
                        out=ps,
                        lhsT=h_sb[:, 2 * kp:2 * kp + 2, pb * P:(pb + 1) * P],
                        rhs=wv[:, kp, :, :],
                        start=(kp == 0), stop=(kp == KP - 1),
                        perf_mode=DR,
                    )
                nc.any.tensor_copy(out=vT_sb[:, pb, :], in_=ps)

            # ---- xb = x + bias_o (residual base) ----
            xb_sb = xbp.tile([P, CCH, F2], BF16, tag="xb")
            for mc in range(CCH):
                nc.any.tensor_scalar_add(
                    out=xb_sb[:, mc, :], in0=x_sb[:, mc, :],
                    scalar1=bo[:, mc:mc + 1],
                )

            # ---- attention per 8-sample group ----
            for gi in range(NGRP):
                c0 = gi * P
                ps_s = psm.tile([P, P], F32, tag="mm")
                for kk in range(CCH):
                    nc.tensor.matmul(
                        out=ps_s,
                        lhsT=g_sb[:, kk, c0:c0 + P],
                        rhs=h_sb[:, kk, c0:c0 + P],
                        start=(kk == 0), stop=(kk == CCH - 1),
                    )
                nc.vector.tensor_add(out=ps_s, in0=ps_s, in1=blkmask)
                p_sb = pp.tile([P, P], F8, tag="pv")
                rs = stp.tile([P, 1], F32, tag="rs")
                nc.scalar.activation(
                    out=p_sb, in_=ps_s, func=AF.Exp, scale=1.0 / SG, accum_out=rs
                )
                rc = stp.tile([P, 1], F32, tag="rc")
                nc.vector.reciprocal(out=rc, in_=rs)
                dg = pp.tile([P, P], F8, tag="dg")
                nc.vector.tensor_scalar_mul(out=dg, in0=ident, scalar1=rc)
                ps_t = psm.tile([P, P], F32, tag="mm")
                nc.tensor.matmul(out=ps_t, lhsT=p_sb, rhs=dg, start=True, stop=True)
                pt_sb = pp.tile([P, P], F8, tag="ptv")
                nc.any.tensor_copy(out=pt_sb, in_=ps_t)
                po = psm.tile([P, 4, P], F32, tag="mm")
                for mc in range(CCH):
                    nc.tensor.matmul(
                        out=po[:, mc, :],
                        lhsT=vT_sb[:, gi, mc * P:(mc + 1) * P], rhs=pt_sb,
                        start=True, stop=True,
                    )
                y_g = yp.tile([P, CCH, P], F32, tag="y")
                for mc in range(CCH):
                    nc.vector.scalar_tensor_tensor(
                        out=y_g[:, mc, :], in0=po[:, mc, :],
                        scalar=1.0 / (SV * SDG), in1=xb_sb[:, mc, c0:c0 + P],
                        op0=A.mult, op1=A.add,
                    )
                nc.sync.dma_start(
                    out=yr[:, :, f0 + c0:f0 + c0 + P], in_=y_g
                )
    return nc


# ---------------------------------------------------------------- host side
def _eff(w, b, gamma, beta, scale=1.0):
    """GN affine folded into conv: W @ (hn*gamma+beta) + b
    = (W*gamma) @ hn + (W@beta + b)."""
    w = np.asarray(w, np.float32)
    b = np.asarray(b, np.float32)
    gamma = np.asarray(gamma, np.float32)
    beta = np.asarray(beta, np.float32)
    return w * gamma[None, :] * scale, (b + w @ beta) * scale


def _pack_w(arr, s):
    """(cin, cout) fp32 -> fp8 [p, kp, 2, cout] with cin=(2*kp+i)*128+p."""
    a = np.clip(arr * s, -240.0, 240.0).astype(F8NP)
    return np.ascontiguousarray(a.reshape(KP, 2, P, C).transpose(2, 0, 1, 3))


def _pack_b(vec, s=1.0):
    return np.ascontiguousarray((np.asarray(vec, np.float32) * s)
                                .reshape(CCH, P).T)


def _fold_phase(wq, bq, wk, bk, wv, bv, wo, bo, gamma, beta):
    scale = float(C) ** -0.5
    aq, bq_e = _eff(wq, bq, gamma, beta, scale)
    ak, bk_e = _eff(wk, bk, gamma, beta)
    av, bv_e = _eff(wv, bv, gamma, beta)
    wo = np.asarray(wo, np.float32)
    bo = np.asarray(bo, np.float32)
    w_g = aq.T @ ak                       # (cin, cout): hn -> g (lhsT layout)
    u_g = ak.T @ bq_e                     # g bias
    n_mat = wo @ av                       # (cout, cin): hn -> v'
    w_v = np.ascontiguousarray(n_mat.T)   # (cin, cout)
    b_o = wo @ bv_e + bo
    return (_pack_w(w_g, SG), _pack_b(u_g, SG),
            _pack_w(w_v, SV), _pack_b(b_o))


def _consts():
    gmask1 = np.zeros((P, GPC), np.float32)
    for p in range(P):
        gmask1[p, p // GS] = 1.0 / GS          # spatial: avg of channel stats
    gmask2 = np.zeros((P, GPC), np.float32)
    for p in range(P):
        gmask2[p, p // GS] = 1.0 / (GS * NT2)  # temporal: full group sum
    bmask = np.zeros((GPC, P), np.float32)
    for p in range(P):
        bmask[p // GS, p] = 1.0
    blk = np.full((P, P), -1e9, np.float32)
    for n in range(P // NT2):
        blk[n * NT2:(n + 1) * NT2, n * NT2:(n + 1) * NT2] = 0.0
    ident = (np.eye(P, dtype=np.float32) * SDG).astype(F8NP)
    return gmask1, gmask2, bmask, ident, blk


_CACHE = {}


def _host_prep(inputs):
    gmask1, gmask2, bmask, ident, blk = _consts()
    wg1, ug1, wv1, bo1 = _fold_phase(
        inputs["wq_s"], inputs["bq_s"], inputs["wk_s"], inputs["bk_s"],
        inputs["wv_s"], inputs["bv_s"], inputs["wo_s"], inputs["bo_s"],
        inputs["gamma_s"], inputs["beta_s"])
    wg2, ug2, wv2, bo2 = _fold_phase(
        inputs["wq_t"], inputs["bq_t"], inputs["wk_t"], inputs["bk_t"],
        inputs["wv_t"], inputs["bv_t"], inputs["wo_t"], inputs["bo_t"],
        inputs["gamma_t"], inputs["beta_t"])
    common1 = dict(wg=wg1, ug=ug1, wv=wv1, bo=bo1,
                   gmask=gmask1, bmask=bmask)
    common2 = dict(wg=wg2, ug=ug2, wv=wv2, bo=bo2,
                   gmask=gmask2.astype(BF16NP), bmask=bmask.astype(BF16NP),
                   blkmask=blk, ident=ident)
    return common1, common2


def _in_maps1(x, common1):
    xs = np.ascontiguousarray(
        np.asarray(x, np.float32).transpose(0, 2, 1, 3, 4)
    ).reshape(B * T, C, L1).astype(BF16NP)
    return [
        dict(xs=np.ascontiguousarray(xs[i * NS1:(i + 1) * NS1]), **common1)
        for i in range(N_CORES)
    ]


def _in_maps2(ys, common2):
    # ys: (B*T, C, L1) bf16 -> (b h w, c, t) -> per-core (C, NS2*T)
    x2 = ys.reshape(B, T, C, H, W).transpose(0, 3, 4, 2, 1)
    x2 = x2.reshape(B * H * W, C, NT2)
    maps = []
    for i in range(N_CORES):
        shard = x2[i * NS2:(i + 1) * NS2]          # (256, 512, 16)
        xt = np.ascontiguousarray(shard.transpose(1, 0, 2)).reshape(C, NS2 * NT2)
        maps.append(dict(xt=xt, **common2))
    return maps


def _assemble(yts):
    out = np.empty((B * H * W, C, NT2), np.float32)
    for i in range(N_CORES):
        yt = yts[i].reshape(C, NS2, NT2)
        out[i * NS2:(i + 1) * NS2] = yt.transpose(1, 0, 2)
    out = out.reshape(B, H, W, C, NT2).transpose(0, 3, 4, 1, 2)
    return np.ascontiguousarray(out)


def kernel(**inputs):
    common1, common2 = _host_prep(inputs)

    if "nc1" not in _CACHE:
        _CACHE["nc1"] = _split_waits(build_spatial())
        _CACHE["nc2"] = _split_waits(build_temporal())
    nc1, nc2 = _CACHE["nc1"], _CACHE["nc2"]

    in_maps1 = _in_maps1(inputs["x"], common1)
    _CACHE["in_maps1"] = in_maps1
    r1 = run_bass_kernel_spmd(nc1, in_maps1, core_ids=list(range(N_CORES)))
    ys = np.concatenate([r1.results[i]["ys"] for i in range(N_CORES)], axis=0)

    in_maps2 = _in_maps2(ys, common2)
    _CACHE["in_maps2"] = in_maps2
    r2 = run_bass_kernel_spmd(nc2, in_maps2, core_ids=list(range(N_CORES)))
    return _assemble([r2.results[i]["yt"] for i in range(N_CORES)])
